# revision 21
# baseline (speedup 1.0000x reference)
"""MoE routing kernel for Trainium2 (8 NeuronCores, expert parallelism).

Strategy
--------
The reference computes a dense MoE (every expert processes every token) and
then combines only the top-2 experts per token.  Mathematically the output
only depends on the tokens each expert was actually selected for, so we:

  host:   router (softmax -> top-2 -> renorm) in float64, aux loss,
          sort tokens by expert id ("all-to-all dispatch" done host-side
          while sharding, as the per-core in_maps are built),
  device: core e runs a dense silu-gated FFN for expert e over the tokens
          routed to it (padded to a static capacity),
  host:   weighted scatter-add combine of the two expert outputs per token.

Device kernel (per core, identical SPMD program):
  x    [H, C]   routed tokens, feature-major
  wg,wu[H, I]   gate/up weights     wd [I, H] down weights
  y    [H, C] = wd.T @ (silu(wg.T @ x + bg) * (wu.T @ x + bu)) + bd

All matmuls run in float32r (~tf32: fp32 storage, 12-bit mantissa rounded by
the PE on read) — 4x the fp32 matmul rate at ~1.5e-4 relative error per
matmul.  Contraction dim on partitions, tokens in the moving-operand free
dim (256..512 per chunk: fp32r needs N>=256 for full rate, <=512 per PSUM
bank).  PSUM accumulates in fp32; biases/activations in fp32.
"""

import numpy as np

E = 8
K = 2
AUX_W = 0.01
B, S, H, I = 2, 2048, 1024, 2048
N = B * S
P = 128
KH = H // P   # 8  contraction tiles for gate/up
KI = I // P   # 16 contraction tiles for down
NCORES = 8

_cache: dict = {}


def _build(cap: int, reps: int = 1):
    """Build + schedule the per-core Bass program for token capacity `cap`.

    reps>1 repeats the whole body (for dispatch-overhead-cancelling timing)."""
    import concourse.bacc as bacc
    import concourse.mybir as mybir
    from concourse import tile

    AF = mybir.ActivationFunctionType
    f32 = mybir.dt.float32
    f32r = mybir.dt.float32r  # ~tf32: 1 cyc/row on PE (4x fp32) when N>=256

    # chunk cap into even pieces within [256, 512] (fp32r needs N>=256 for
    # full rate and even innermost free size); cap is even and >= 256
    assert cap >= 256 and cap % 2 == 0
    sizes = []
    rem = cap
    while rem >= 768:
        sizes.append(512)
        rem -= 512
    if rem > 512:
        sizes.extend([rem - 256, 256])
    else:
        sizes.append(rem)
    c_chunks = []
    c0 = 0
    for cl in sizes:
        c_chunks.append((c0, cl))
        c0 += cl

    nc = bacc.Bacc("TRN2", target_bir_lowering=False, debug=False)
    xt = nc.dram_tensor("xt", [H, cap], f32r, kind="ExternalInput")
    wg = nc.dram_tensor("wg", [H, I], f32r, kind="ExternalInput")
    wu = nc.dram_tensor("wu", [H, I], f32r, kind="ExternalInput")
    wd = nc.dram_tensor("wd", [I, H], f32r, kind="ExternalInput")
    bg = nc.dram_tensor("bg", [I], f32, kind="ExternalInput")
    bu = nc.dram_tensor("bu", [I], f32, kind="ExternalInput")
    bd = nc.dram_tensor("bd", [H], f32, kind="ExternalInput")
    yt = nc.dram_tensor("yt", [H, cap], f32, kind="ExternalOutput")

    xt_r = xt.ap().rearrange("(k p) c -> p k c", p=P)    # [128, KH, cap]
    wd_r = wd.ap().rearrange("(k p) h -> p k h", p=P)    # [128, KI, H]
    yt_ap = yt.ap()

    with tile.TileContext(nc) as tc:
        with (
            tc.tile_pool(name="xp", bufs=1) as xp,
            tc.tile_pool(name="gup", bufs=1) as gup,
            tc.tile_pool(name="wp", bufs=2) as wp,
            tc.tile_pool(name="wdp", bufs=2) as wdp,
            tc.tile_pool(name="bp", bufs=1) as bp,
            tc.tile_pool(name="actp", bufs=4) as actp,
            tc.tile_pool(name="yp", bufs=3) as yp,
            tc.tile_pool(name="ps", bufs=3, space="PSUM") as ps,
            tc.tile_pool(name="psy", bufs=2, space="PSUM") as psy,
        ):
          for _rep in range(reps):
            # --- resident inputs ---
            # one tile per kh slice so the first matmuls only wait for the
            # first slice's DMA, not all of x
            x_sb = [
                xp.tile([P, 1, cap], f32r, name=f"x_sb{k}", tag=f"x_sb{k}")
                for k in range(KH)
            ]
            for k in range(KH):
                nc.sync.dma_start(x_sb[k][:], xt_r[:, k : k + 1, :])

            bg_sb = bp.tile([P, KI], f32, tag="bg", name="bg_sb")
            nc.sync.dma_start(bg_sb[:], bg.ap().rearrange("(k p) -> p k", p=P))
            bu_sb = bp.tile([P, KI], f32, tag="bu", name="bu_sb")
            nc.sync.dma_start(bu_sb[:], bu.ap().rearrange("(k p) -> p k", p=P))
            bd_sb = bp.tile([P, KH], f32, tag="bd", name="bd_sb")
            nc.sync.dma_start(bd_sb[:], bd.ap().rearrange("(k p) -> p k", p=P))

            gu_sb = gup.tile([P, KI, cap], f32r, name="gu_sb", tag="gu_sb")

            # --- phase 1: gate & up projections + silu + mul ---
            # weights DMA'd two i-chunks (256 cols) at a time for 1KB lines
            for ki in range(KI):
                i0 = ki * P
                if ki % 2 == 0:
                    wg_t2 = wp.tile([P, KH, 2 * P], f32r, tag="wg", name="wg_t2")
                    nc.sync.dma_start(
                        wg_t2[:],
                        wg.ap()[:, i0 : i0 + 2 * P].rearrange(
                            "(k p) i -> p k i", p=P
                        ),
                    )
                    wu_t2 = wp.tile([P, KH, 2 * P], f32r, tag="wu", name="wu_t2")
                    nc.sync.dma_start(
                        wu_t2[:],
                        wu.ap()[:, i0 : i0 + 2 * P].rearrange(
                            "(k p) i -> p k i", p=P
                        ),
                    )
                half = slice((ki % 2) * P, (ki % 2) * P + P)
                wg_t = wg_t2[:, :, half]
                wu_t = wu_t2[:, :, half]
                for c0, cl in c_chunks:
                    g_ps = ps.tile([P, 512], f32, tag="g_ps", name="g_ps")[:, :cl]
                    for kh in range(KH):
                        nc.tensor.matmul(
                            g_ps,
                            lhsT=wg_t[:, kh : kh + 1, :],
                            rhs=x_sb[kh][:, :, c0 : c0 + cl],
                            start=(kh == 0),
                            stop=(kh == KH - 1),
                        )
                    u_ps = ps.tile([P, 512], f32, tag="u_ps", name="u_ps")[:, :cl]
                    for kh in range(KH):
                        nc.tensor.matmul(
                            u_ps,
                            lhsT=wu_t[:, kh : kh + 1, :],
                            rhs=x_sb[kh][:, :, c0 : c0 + cl],
                            start=(kh == 0),
                            stop=(kh == KH - 1),
                        )
                    g_act = actp.tile([P, 512], f32, tag="g_act", name="g_act")[:, :cl]
                    nc.scalar.activation(
                        g_act, g_ps, AF.Silu, bias=bg_sb[:, ki : ki + 1]
                    )
                    # gu = (u_ps + bu) * silu(g_ps + bg), rounded to f32r by DVE
                    nc.vector.scalar_tensor_tensor(
                        gu_sb[:, ki : ki + 1, c0 : c0 + cl],
                        u_ps,
                        bu_sb[:, ki : ki + 1],
                        g_act,
                        mybir.AluOpType.add,
                        mybir.AluOpType.mult,
                    )

            # --- phase 2: down projection (weights 2 h-chunks per DMA) ---
            for m in range(KH):
                h0 = m * P
                if m % 2 == 0:
                    wd_t2 = wdp.tile([P, KI, 2 * P], f32r, tag="wd", name="wd_t2")
                    nc.sync.dma_start(wd_t2[:], wd_r[:, :, h0 : h0 + 2 * P])
                wd_t = wd_t2[:, :, slice((m % 2) * P, (m % 2) * P + P)]
                for c0, cl in c_chunks:
                    y_ps = psy.tile([P, 512], f32, tag="y_ps", name="y_ps")[:, :cl]
                    for ki in range(KI):
                        nc.tensor.matmul(
                            y_ps,
                            lhsT=wd_t[:, ki : ki + 1, :],
                            rhs=gu_sb[:, ki : ki + 1, c0 : c0 + cl],
                            start=(ki == 0),
                            stop=(ki == KI - 1),
                        )
                    y_sb = yp.tile([P, 512], f32, tag="y_sb", name="y_sb")[:, :cl]
                    nc.scalar.activation(
                        y_sb, y_ps, AF.Identity, bias=bd_sb[:, m : m + 1]
                    )
                    nc.sync.dma_start(yt_ap[h0 : h0 + P, c0 : c0 + cl], y_sb)

    nc.compile()
    return nc


def _route(xf):
    """float64 router: softmax -> top-2 (jax.lax.top_k tie order) -> renorm."""
    logits = xf.astype(np.float64) @ _route.wr64
    logits -= logits.max(-1, keepdims=True)
    p = np.exp(logits)
    p /= p.sum(-1, keepdims=True)
    sel = np.argsort(-p, axis=-1, kind="stable")[:, :K]
    rw = np.take_along_axis(p, sel, axis=-1)
    rw = rw / rw.sum(-1, keepdims=True)
    return sel, rw, p


def prepare(hidden_states, w_router, wg, bg, wu, bu, wd, bd):
    """Host routing + dispatch: returns (in_maps, idxs, ws, counts, cap, aux)."""
    hidden_states = np.asarray(hidden_states)
    w_router = np.asarray(w_router)
    xf = np.ascontiguousarray(hidden_states.reshape(N, H), dtype=np.float32)

    # ---- host routing ----
    _route.wr64 = w_router.astype(np.float64).T
    sel, rw64, _ = _route(xf)
    rw = rw64.astype(np.float32)

    # aux loss (float64, cast at the end, mirrors reference formula)
    rppe_sum = np.zeros(E)
    for k in range(K):
        np.add.at(rppe_sum, sel[:, k], rw64[:, k])
    aux_loss = np.float32(
        np.mean(rppe_sum * (rppe_sum / N)) * E * AUX_W
    )

    # ---- dispatch: sort tokens by expert ----
    idxs, ws = [], []
    for e in range(E):
        m0 = sel[:, 0] == e
        m1 = sel[:, 1] == e
        idx = np.concatenate([np.nonzero(m0)[0], np.nonzero(m1)[0]])
        w = np.concatenate([rw[m0, 0], rw[m1, 1]])
        idxs.append(idx)
        ws.append(w)
    counts = [len(i) for i in idxs]
    cap = max(256, -(-max(counts) // 2) * 2)  # even (fp32r free-dim rule)

    in_maps = _pack(xf, idxs, counts, cap, wg, bg, wu, bu, wd, bd)
    return in_maps, idxs, ws, counts, cap, aux_loss


def _pack(xf, idxs, counts, cap, wg, bg, wu, bu, wd, bd):
    in_maps = []
    for e in range(E):
        xt = np.zeros((H, cap), np.float32)
        xt[:, : counts[e]] = xf[idxs[e]].T
        in_maps.append(
            {
                "xt": xt,
                "wg": np.ascontiguousarray(np.asarray(wg)[e], dtype=np.float32),
                "wu": np.ascontiguousarray(np.asarray(wu)[e], dtype=np.float32),
                "wd": np.ascontiguousarray(np.asarray(wd)[e], dtype=np.float32),
                "bg": np.ascontiguousarray(np.asarray(bg)[e], dtype=np.float32),
                "bu": np.ascontiguousarray(np.asarray(bu)[e], dtype=np.float32),
                "bd": np.ascontiguousarray(np.asarray(bd)[e], dtype=np.float32),
            }
        )
    return in_maps


def combine(results, idxs, ws, counts):
    acc = np.zeros((N, H), np.float32)
    for e in range(E):
        y = results[e]["yt"][:, : counts[e]].T  # [cnt, H]
        acc[idxs[e]] += y * ws[e][:, None]
    return acc.reshape(B, S, H)


CAP_MAX = 1280  # largest per-round capacity that fits the SBUF layout


def kernel(hidden_states, w_router, wg, bg, wu, bu, wd, bd):
    from concourse.bass_utils import run_bass_kernel_spmd

    in_maps, idxs, ws, counts, cap, aux_loss = prepare(
        hidden_states, w_router, wg, bg, wu, bu, wd, bd
    )
    cores = list(range(NCORES))

    if cap <= CAP_MAX:
        if cap not in _cache:
            _cache[cap] = _build(cap)
        res = run_bass_kernel_spmd(_cache[cap], in_maps, core_ids=cores)
        return combine(res.results, idxs, ws, counts), aux_loss

    # extreme routing skew: process each expert's tokens in multiple rounds
    xf = np.ascontiguousarray(
        np.asarray(hidden_states).reshape(N, H), dtype=np.float32
    )
    acc = np.zeros((N, H), np.float32)
    rounds = -(-max(counts) // CAP_MAX)
    for r in range(rounds):
        sub_idxs = [ix[r * CAP_MAX : (r + 1) * CAP_MAX] for ix in idxs]
        sub_ws = [w[r * CAP_MAX : (r + 1) * CAP_MAX] for w in ws]
        sub_counts = [len(ix) for ix in sub_idxs]
        cap_r = max(256, -(-max(sub_counts) // 2) * 2)
        if cap_r not in _cache:
            _cache[cap_r] = _build(cap_r)
        maps_r = _pack(xf, sub_idxs, sub_counts, cap_r, wg, bg, wu, bu, wd, bd)
        res = run_bass_kernel_spmd(_cache[cap_r], maps_r, core_ids=cores)
        acc += combine(res.results, sub_idxs, sub_ws, sub_counts).reshape(N, H)
    return acc.reshape(B, S, H), aux_loss


# revision 22
# speedup vs baseline: 1.1429x; 1.1429x over previous
"""MoE routing kernel for Trainium2 (8 NeuronCores, expert parallelism).

Strategy
--------
The reference computes a dense MoE (every expert processes every token) and
then combines only the top-2 experts per token.  Mathematically the output
only depends on the tokens each expert was actually selected for, so we:

  host:   router (softmax -> top-2 -> renorm) in float64, aux loss,
          sort tokens by expert id ("all-to-all dispatch" done host-side
          while sharding, as the per-core in_maps are built),
  device: core e runs a dense silu-gated FFN for expert e over the tokens
          routed to it (padded to a static capacity),
  host:   weighted scatter-add combine of the two expert outputs per token.

Device kernel (per core, identical SPMD program):
  x    [H, C]   routed tokens, feature-major
  wg,wu[H, I]   gate/up weights     wd [I, H] down weights
  y    [H, C] = wd.T @ (silu(wg.T @ x + bg) * (wu.T @ x + bu)) + bd

All matmuls run in float32r (~tf32: fp32 storage, 12-bit mantissa rounded by
the PE on read) — 4x the fp32 matmul rate at ~1.5e-4 relative error per
matmul.  Contraction dim on partitions, tokens in the moving-operand free
dim (256..512 per chunk: fp32r needs N>=256 for full rate, <=512 per PSUM
bank).  PSUM accumulates in fp32; biases/activations in fp32.
"""

import numpy as np

E = 8
K = 2
AUX_W = 0.01
B, S, H, I = 2, 2048, 1024, 2048
N = B * S
P = 128
KH = H // P   # 8  contraction tiles for gate/up
KI = I // P   # 16 contraction tiles for down
NCORES = 8

_cache: dict = {}


def _build(cap: int, reps: int = 1):
    """Build + schedule the per-core Bass program for token capacity `cap`.

    reps>1 repeats the whole body (for dispatch-overhead-cancelling timing)."""
    import concourse.bacc as bacc
    import concourse.mybir as mybir
    from concourse import tile

    AF = mybir.ActivationFunctionType
    f32 = mybir.dt.float32
    f32r = mybir.dt.float32r  # ~tf32: 1 cyc/row on PE (4x fp32) when N>=256

    # chunk cap into even pieces within [256, 512] (fp32r needs N>=256 for
    # full rate and even innermost free size); cap is even and >= 256
    assert cap >= 256 and cap % 2 == 0
    sizes = []
    rem = cap
    while rem >= 768:
        sizes.append(512)
        rem -= 512
    if rem > 512:
        sizes.extend([rem - 256, 256])
    else:
        sizes.append(rem)
    c_chunks = []
    c0 = 0
    for cl in sizes:
        c_chunks.append((c0, cl))
        c0 += cl

    nc = bacc.Bacc("TRN2", target_bir_lowering=False, debug=False)
    xt = nc.dram_tensor("xt", [H, cap], f32r, kind="ExternalInput")
    # weights arrive host-retiled: block j is one contiguous DMA with 8KB
    # per-partition lines: wg/wu [KI//2, P, KH, 256], wd [KH//2, P, KI, 256]
    wg = nc.dram_tensor("wg", [KI // 2, P, KH, 2 * P], f32r, kind="ExternalInput")
    wu = nc.dram_tensor("wu", [KI // 2, P, KH, 2 * P], f32r, kind="ExternalInput")
    wd = nc.dram_tensor("wd", [KH // 2, P, KI, 2 * P], f32r, kind="ExternalInput")
    bg = nc.dram_tensor("bg", [I], f32, kind="ExternalInput")
    bu = nc.dram_tensor("bu", [I], f32, kind="ExternalInput")
    bd = nc.dram_tensor("bd", [H], f32, kind="ExternalInput")
    yt = nc.dram_tensor("yt", [H, cap], f32, kind="ExternalOutput")

    xt_r = xt.ap().rearrange("(k p) c -> p k c", p=P)    # [128, KH, cap]
    yt_ap = yt.ap()

    with tile.TileContext(nc) as tc:
        with (
            tc.tile_pool(name="xp", bufs=1) as xp,
            tc.tile_pool(name="gup", bufs=1) as gup,
            tc.tile_pool(name="wp", bufs=2) as wp,
            tc.tile_pool(name="wdp", bufs=2) as wdp,
            tc.tile_pool(name="bp", bufs=1) as bp,
            tc.tile_pool(name="actp", bufs=4) as actp,
            tc.tile_pool(name="yp", bufs=3) as yp,
            tc.tile_pool(name="ps", bufs=3, space="PSUM") as ps,
            tc.tile_pool(name="psy", bufs=2, space="PSUM") as psy,
        ):
          for _rep in range(reps):
            # --- resident inputs ---
            # one tile per kh slice so the first matmuls only wait for the
            # first slice's DMA, not all of x
            x_sb = [
                xp.tile([P, 1, cap], f32r, name=f"x_sb{k}", tag=f"x_sb{k}")
                for k in range(KH)
            ]
            for k in range(KH):
                nc.sync.dma_start(x_sb[k][:], xt_r[:, k : k + 1, :])

            bg_sb = bp.tile([P, KI], f32, tag="bg", name="bg_sb")
            nc.sync.dma_start(bg_sb[:], bg.ap().rearrange("(k p) -> p k", p=P))
            bu_sb = bp.tile([P, KI], f32, tag="bu", name="bu_sb")
            nc.sync.dma_start(bu_sb[:], bu.ap().rearrange("(k p) -> p k", p=P))
            bd_sb = bp.tile([P, KH], f32, tag="bd", name="bd_sb")
            nc.sync.dma_start(bd_sb[:], bd.ap().rearrange("(k p) -> p k", p=P))

            gu_sb = gup.tile([P, KI, cap], f32r, name="gu_sb", tag="gu_sb")

            # --- phase 1: gate & up projections + silu + mul ---
            # weights DMA'd two i-chunks (256 cols) at a time for 1KB lines
            for ki in range(KI):
                i0 = ki * P
                if ki % 2 == 0:
                    wg_t2 = wp.tile([P, KH, 2 * P], f32r, tag="wg", name="wg_t2")
                    nc.sync.dma_start(wg_t2[:], wg.ap()[ki // 2])
                    wu_t2 = wp.tile([P, KH, 2 * P], f32r, tag="wu", name="wu_t2")
                    nc.sync.dma_start(wu_t2[:], wu.ap()[ki // 2])
                half = slice((ki % 2) * P, (ki % 2) * P + P)
                wg_t = wg_t2[:, :, half]
                wu_t = wu_t2[:, :, half]
                for c0, cl in c_chunks:
                    g_ps = ps.tile([P, 512], f32, tag="g_ps", name="g_ps")[:, :cl]
                    for kh in range(KH):
                        nc.tensor.matmul(
                            g_ps,
                            lhsT=wg_t[:, kh : kh + 1, :],
                            rhs=x_sb[kh][:, :, c0 : c0 + cl],
                            start=(kh == 0),
                            stop=(kh == KH - 1),
                        )
                    u_ps = ps.tile([P, 512], f32, tag="u_ps", name="u_ps")[:, :cl]
                    for kh in range(KH):
                        nc.tensor.matmul(
                            u_ps,
                            lhsT=wu_t[:, kh : kh + 1, :],
                            rhs=x_sb[kh][:, :, c0 : c0 + cl],
                            start=(kh == 0),
                            stop=(kh == KH - 1),
                        )
                    g_act = actp.tile([P, 512], f32, tag="g_act", name="g_act")[:, :cl]
                    nc.scalar.activation(
                        g_act, g_ps, AF.Silu, bias=bg_sb[:, ki : ki + 1]
                    )
                    # gu = (u_ps + bu) * silu(g_ps + bg), rounded to f32r by DVE
                    nc.vector.scalar_tensor_tensor(
                        gu_sb[:, ki : ki + 1, c0 : c0 + cl],
                        u_ps,
                        bu_sb[:, ki : ki + 1],
                        g_act,
                        mybir.AluOpType.add,
                        mybir.AluOpType.mult,
                    )

            # --- phase 2: down projection (weights 2 h-chunks per DMA) ---
            for m in range(KH):
                h0 = m * P
                if m % 2 == 0:
                    wd_t2 = wdp.tile([P, KI, 2 * P], f32r, tag="wd", name="wd_t2")
                    nc.sync.dma_start(wd_t2[:], wd.ap()[m // 2])
                wd_t = wd_t2[:, :, slice((m % 2) * P, (m % 2) * P + P)]
                for c0, cl in c_chunks:
                    y_ps = psy.tile([P, 512], f32, tag="y_ps", name="y_ps")[:, :cl]
                    for ki in range(KI):
                        nc.tensor.matmul(
                            y_ps,
                            lhsT=wd_t[:, ki : ki + 1, :],
                            rhs=gu_sb[:, ki : ki + 1, c0 : c0 + cl],
                            start=(ki == 0),
                            stop=(ki == KI - 1),
                        )
                    y_sb = yp.tile([P, 512], f32, tag="y_sb", name="y_sb")[:, :cl]
                    nc.scalar.activation(
                        y_sb, y_ps, AF.Identity, bias=bd_sb[:, m : m + 1]
                    )
                    nc.sync.dma_start(yt_ap[h0 : h0 + P, c0 : c0 + cl], y_sb)

    nc.compile()
    return nc


def _route(xf):
    """float64 router: softmax -> top-2 (jax.lax.top_k tie order) -> renorm."""
    logits = xf.astype(np.float64) @ _route.wr64
    logits -= logits.max(-1, keepdims=True)
    p = np.exp(logits)
    p /= p.sum(-1, keepdims=True)
    sel = np.argsort(-p, axis=-1, kind="stable")[:, :K]
    rw = np.take_along_axis(p, sel, axis=-1)
    rw = rw / rw.sum(-1, keepdims=True)
    return sel, rw, p


def prepare(hidden_states, w_router, wg, bg, wu, bu, wd, bd):
    """Host routing + dispatch: returns (in_maps, idxs, ws, counts, cap, aux)."""
    hidden_states = np.asarray(hidden_states)
    w_router = np.asarray(w_router)
    xf = np.ascontiguousarray(hidden_states.reshape(N, H), dtype=np.float32)

    # ---- host routing ----
    _route.wr64 = w_router.astype(np.float64).T
    sel, rw64, _ = _route(xf)
    rw = rw64.astype(np.float32)

    # aux loss (float64, cast at the end, mirrors reference formula)
    rppe_sum = np.zeros(E)
    for k in range(K):
        np.add.at(rppe_sum, sel[:, k], rw64[:, k])
    aux_loss = np.float32(
        np.mean(rppe_sum * (rppe_sum / N)) * E * AUX_W
    )

    # ---- dispatch: sort tokens by expert ----
    idxs, ws = [], []
    for e in range(E):
        m0 = sel[:, 0] == e
        m1 = sel[:, 1] == e
        idx = np.concatenate([np.nonzero(m0)[0], np.nonzero(m1)[0]])
        w = np.concatenate([rw[m0, 0], rw[m1, 1]])
        idxs.append(idx)
        ws.append(w)
    counts = [len(i) for i in idxs]
    cap = max(256, -(-max(counts) // 2) * 2)  # even (fp32r free-dim rule)

    in_maps = _pack(xf, idxs, counts, cap, wg, bg, wu, bu, wd, bd)
    return in_maps, idxs, ws, counts, cap, aux_loss


def _retile_up(w):
    """[H, I] -> [KI//2, P, KH, 256]: block j contiguous, 8KB partition lines."""
    w = np.asarray(w, dtype=np.float32)
    return np.ascontiguousarray(
        w.reshape(KH, P, KI // 2, 2 * P).transpose(2, 1, 0, 3)
    )


def _retile_dn(w):
    """[I, H] -> [KH//2, P, KI, 256]."""
    w = np.asarray(w, dtype=np.float32)
    return np.ascontiguousarray(
        w.reshape(KI, P, KH // 2, 2 * P).transpose(2, 1, 0, 3)
    )


def _pack(xf, idxs, counts, cap, wg, bg, wu, bu, wd, bd):
    in_maps = []
    for e in range(E):
        xt = np.zeros((H, cap), np.float32)
        xt[:, : counts[e]] = xf[idxs[e]].T
        in_maps.append(
            {
                "xt": xt,
                "wg": _retile_up(np.asarray(wg)[e]),
                "wu": _retile_up(np.asarray(wu)[e]),
                "wd": _retile_dn(np.asarray(wd)[e]),
                "bg": np.ascontiguousarray(np.asarray(bg)[e], dtype=np.float32),
                "bu": np.ascontiguousarray(np.asarray(bu)[e], dtype=np.float32),
                "bd": np.ascontiguousarray(np.asarray(bd)[e], dtype=np.float32),
            }
        )
    return in_maps


def combine(results, idxs, ws, counts):
    acc = np.zeros((N, H), np.float32)
    for e in range(E):
        y = results[e]["yt"][:, : counts[e]].T  # [cnt, H]
        acc[idxs[e]] += y * ws[e][:, None]
    return acc.reshape(B, S, H)


CAP_MAX = 1280  # largest per-round capacity that fits the SBUF layout


def kernel(hidden_states, w_router, wg, bg, wu, bu, wd, bd):
    from concourse.bass_utils import run_bass_kernel_spmd

    in_maps, idxs, ws, counts, cap, aux_loss = prepare(
        hidden_states, w_router, wg, bg, wu, bu, wd, bd
    )
    cores = list(range(NCORES))

    if cap <= CAP_MAX:
        if cap not in _cache:
            _cache[cap] = _build(cap)
        res = run_bass_kernel_spmd(_cache[cap], in_maps, core_ids=cores)
        return combine(res.results, idxs, ws, counts), aux_loss

    # extreme routing skew: process each expert's tokens in multiple rounds
    xf = np.ascontiguousarray(
        np.asarray(hidden_states).reshape(N, H), dtype=np.float32
    )
    acc = np.zeros((N, H), np.float32)
    rounds = -(-max(counts) // CAP_MAX)
    for r in range(rounds):
        sub_idxs = [ix[r * CAP_MAX : (r + 1) * CAP_MAX] for ix in idxs]
        sub_ws = [w[r * CAP_MAX : (r + 1) * CAP_MAX] for w in ws]
        sub_counts = [len(ix) for ix in sub_idxs]
        cap_r = max(256, -(-max(sub_counts) // 2) * 2)
        if cap_r not in _cache:
            _cache[cap_r] = _build(cap_r)
        maps_r = _pack(xf, sub_idxs, sub_counts, cap_r, wg, bg, wu, bu, wd, bd)
        res = run_bass_kernel_spmd(_cache[cap_r], maps_r, core_ids=cores)
        acc += combine(res.results, sub_idxs, sub_ws, sub_counts).reshape(N, H)
    return acc.reshape(B, S, H), aux_loss


# revision 23
# speedup vs baseline: 3.0704x; 2.6865x over previous
"""MoE routing kernel for Trainium2 (8 NeuronCores, expert parallelism).

Strategy
--------
The reference computes a dense MoE (every expert processes every token) and
then combines only the top-2 experts per token.  Mathematically the output
only depends on the tokens each expert was actually selected for, so we:

  host:   router (softmax -> top-2 -> renorm) in float64, aux loss,
          sort tokens by expert id ("all-to-all dispatch" done host-side
          while sharding, as the per-core in_maps are built),
  device: core e runs a dense silu-gated FFN for expert e over the tokens
          routed to it (padded to a static capacity),
  host:   weighted scatter-add combine of the two expert outputs per token.

Device kernel (per core, identical SPMD program):
  x    [H, C]   routed tokens, feature-major
  wg,wu[H, I]   gate/up weights     wd [I, H] down weights
  y    [H, C] = wd.T @ (silu(wg.T @ x + bg) * (wu.T @ x + bu)) + bd

All matmuls run in float32r (~tf32: fp32 storage, 12-bit mantissa rounded by
the PE on read) — 4x the fp32 matmul rate at ~1.5e-4 relative error per
matmul.  Contraction dim on partitions, tokens in the moving-operand free
dim (256..512 per chunk: fp32r needs N>=256 for full rate, <=512 per PSUM
bank).  PSUM accumulates in fp32; biases/activations in fp32.
"""

import numpy as np

E = 8
K = 2
AUX_W = 0.01
B, S, H, I = 2, 2048, 1024, 2048
N = B * S
P = 128
KH = H // P   # 8  contraction tiles for gate/up
KI = I // P   # 16 contraction tiles for down
NCORES = 8

_cache: dict = {}


def _build(cap: int, reps: int = 1):
    """Build + schedule the per-core Bass program for token capacity `cap`.

    reps>1 repeats the whole body (for dispatch-overhead-cancelling timing)."""
    import concourse.bacc as bacc
    import concourse.mybir as mybir
    from concourse import tile

    AF = mybir.ActivationFunctionType
    f32 = mybir.dt.float32
    f32r = mybir.dt.float32r  # ~tf32: 1 cyc/row on PE (4x fp32) when N>=256

    # chunk cap into even pieces within [256, 512] (fp32r needs N>=256 for
    # full rate and even innermost free size); cap is even and >= 256
    assert cap >= 256 and cap % 2 == 0
    sizes = []
    rem = cap
    while rem >= 768:
        sizes.append(512)
        rem -= 512
    if rem > 512:
        sizes.extend([rem - 256, 256])
    else:
        sizes.append(rem)
    c_chunks = []
    c0 = 0
    for cl in sizes:
        c_chunks.append((c0, cl))
        c0 += cl

    nc = bacc.Bacc("TRN2", target_bir_lowering=False, debug=False)
    xt = nc.dram_tensor("xt", [H, cap], f32r, kind="ExternalInput")
    # weights arrive host-retiled: block j is one contiguous DMA with 8KB
    # per-partition lines: wg/wu [KI//2, P, KH, 256], wd [KH//2, P, KI, 256]
    wg = nc.dram_tensor("wg", [KI // 2, P, KH, 2 * P], f32r, kind="ExternalInput")
    wu = nc.dram_tensor("wu", [KI // 2, P, KH, 2 * P], f32r, kind="ExternalInput")
    wd = nc.dram_tensor("wd", [KH // 2, P, KI, 2 * P], f32r, kind="ExternalInput")
    bg = nc.dram_tensor("bg", [I], f32, kind="ExternalInput")
    bu = nc.dram_tensor("bu", [I], f32, kind="ExternalInput")
    bd = nc.dram_tensor("bd", [H], f32, kind="ExternalInput")
    yt = nc.dram_tensor("yt", [H, cap], f32, kind="ExternalOutput")

    xt_r = xt.ap().rearrange("(k p) c -> p k c", p=P)    # [128, KH, cap]
    yt_ap = yt.ap()

    with tile.TileContext(nc) as tc:
        with (
            tc.tile_pool(name="xp", bufs=1) as xp,
            tc.tile_pool(name="gup", bufs=1) as gup,
            tc.tile_pool(name="wp", bufs=2) as wp,
            tc.tile_pool(name="wdp", bufs=2) as wdp,
            tc.tile_pool(name="bp", bufs=1) as bp,
            tc.tile_pool(name="actp", bufs=4) as actp,
            tc.tile_pool(name="yp", bufs=3) as yp,
            tc.tile_pool(name="ps", bufs=3, space="PSUM") as ps,
            tc.tile_pool(name="psy", bufs=2, space="PSUM") as psy,
        ):
          for _rep in range(reps):
            # --- resident inputs ---
            # tiny bias DMAs first: the HWDGE ring drains FIFO, and the first
            # silu needs bg — don't queue it behind 4.4MB of x slices
            bg_sb = bp.tile([P, KI], f32, tag="bg", name="bg_sb")
            nc.sync.dma_start(bg_sb[:], bg.ap().rearrange("(k p) -> p k", p=P))
            bu_sb = bp.tile([P, KI], f32, tag="bu", name="bu_sb")
            nc.sync.dma_start(bu_sb[:], bu.ap().rearrange("(k p) -> p k", p=P))
            bd_sb = bp.tile([P, KH], f32, tag="bd", name="bd_sb")
            nc.sync.dma_start(bd_sb[:], bd.ap().rearrange("(k p) -> p k", p=P))

            # one tile per kh slice so the first matmuls only wait for the
            # first slice's DMA, not all of x
            x_sb = [
                xp.tile([P, 1, cap], f32r, name=f"x_sb{k}", tag=f"x_sb{k}")
                for k in range(KH)
            ]
            for k in range(KH):
                nc.sync.dma_start(x_sb[k][:], xt_r[:, k : k + 1, :])

            gu_sb = gup.tile([P, KI, cap], f32r, name="gu_sb", tag="gu_sb")

            # --- phase 1: gate & up projections + silu + mul ---
            # weights DMA'd two i-chunks (256 cols) at a time for 1KB lines
            for ki in range(KI):
                i0 = ki * P
                if ki % 2 == 0:
                    wg_t2 = wp.tile([P, KH, 2 * P], f32r, tag="wg", name="wg_t2")
                    nc.sync.dma_start(wg_t2[:], wg.ap()[ki // 2])
                    wu_t2 = wp.tile([P, KH, 2 * P], f32r, tag="wu", name="wu_t2")
                    nc.sync.dma_start(wu_t2[:], wu.ap()[ki // 2])
                half = slice((ki % 2) * P, (ki % 2) * P + P)
                wg_t = wg_t2[:, :, half]
                wu_t = wu_t2[:, :, half]
                for c0, cl in c_chunks:
                    g_ps = ps.tile([P, 512], f32, tag="g_ps", name="g_ps")[:, :cl]
                    for kh in range(KH):
                        nc.tensor.matmul(
                            g_ps,
                            lhsT=wg_t[:, kh : kh + 1, :],
                            rhs=x_sb[kh][:, :, c0 : c0 + cl],
                            start=(kh == 0),
                            stop=(kh == KH - 1),
                        )
                    u_ps = ps.tile([P, 512], f32, tag="u_ps", name="u_ps")[:, :cl]
                    for kh in range(KH):
                        nc.tensor.matmul(
                            u_ps,
                            lhsT=wu_t[:, kh : kh + 1, :],
                            rhs=x_sb[kh][:, :, c0 : c0 + cl],
                            start=(kh == 0),
                            stop=(kh == KH - 1),
                        )
                    g_act = actp.tile([P, 512], f32, tag="g_act", name="g_act")[:, :cl]
                    nc.scalar.activation(
                        g_act, g_ps, AF.Silu, bias=bg_sb[:, ki : ki + 1]
                    )
                    # gu = (u_ps + bu) * silu(g_ps + bg), rounded to f32r by DVE
                    nc.vector.scalar_tensor_tensor(
                        gu_sb[:, ki : ki + 1, c0 : c0 + cl],
                        u_ps,
                        bu_sb[:, ki : ki + 1],
                        g_act,
                        mybir.AluOpType.add,
                        mybir.AluOpType.mult,
                    )

            # --- phase 2: down projection (weights 2 h-chunks per DMA) ---
            for m in range(KH):
                h0 = m * P
                if m % 2 == 0:
                    wd_t2 = wdp.tile([P, KI, 2 * P], f32r, tag="wd", name="wd_t2")
                    nc.sync.dma_start(wd_t2[:], wd.ap()[m // 2])
                wd_t = wd_t2[:, :, slice((m % 2) * P, (m % 2) * P + P)]
                for c0, cl in c_chunks:
                    y_ps = psy.tile([P, 512], f32, tag="y_ps", name="y_ps")[:, :cl]
                    for ki in range(KI):
                        nc.tensor.matmul(
                            y_ps,
                            lhsT=wd_t[:, ki : ki + 1, :],
                            rhs=gu_sb[:, ki : ki + 1, c0 : c0 + cl],
                            start=(ki == 0),
                            stop=(ki == KI - 1),
                        )
                    y_sb = yp.tile([P, 512], f32, tag="y_sb", name="y_sb")[:, :cl]
                    nc.scalar.activation(
                        y_sb, y_ps, AF.Identity, bias=bd_sb[:, m : m + 1]
                    )
                    nc.sync.dma_start(yt_ap[h0 : h0 + P, c0 : c0 + cl], y_sb)

    nc.compile()
    return nc


def _route(xf):
    """float64 router: softmax -> top-2 (jax.lax.top_k tie order) -> renorm."""
    logits = xf.astype(np.float64) @ _route.wr64
    logits -= logits.max(-1, keepdims=True)
    p = np.exp(logits)
    p /= p.sum(-1, keepdims=True)
    sel = np.argsort(-p, axis=-1, kind="stable")[:, :K]
    rw = np.take_along_axis(p, sel, axis=-1)
    rw = rw / rw.sum(-1, keepdims=True)
    return sel, rw, p


def prepare(hidden_states, w_router, wg, bg, wu, bu, wd, bd):
    """Host routing + dispatch: returns (in_maps, idxs, ws, counts, cap, aux)."""
    hidden_states = np.asarray(hidden_states)
    w_router = np.asarray(w_router)
    xf = np.ascontiguousarray(hidden_states.reshape(N, H), dtype=np.float32)

    # ---- host routing ----
    _route.wr64 = w_router.astype(np.float64).T
    sel, rw64, _ = _route(xf)
    rw = rw64.astype(np.float32)

    # aux loss (float64, cast at the end, mirrors reference formula)
    rppe_sum = np.zeros(E)
    for k in range(K):
        np.add.at(rppe_sum, sel[:, k], rw64[:, k])
    aux_loss = np.float32(
        np.mean(rppe_sum * (rppe_sum / N)) * E * AUX_W
    )

    # ---- dispatch: sort tokens by expert ----
    idxs, ws = [], []
    for e in range(E):
        m0 = sel[:, 0] == e
        m1 = sel[:, 1] == e
        idx = np.concatenate([np.nonzero(m0)[0], np.nonzero(m1)[0]])
        w = np.concatenate([rw[m0, 0], rw[m1, 1]])
        idxs.append(idx)
        ws.append(w)
    counts = [len(i) for i in idxs]
    cap = max(256, -(-max(counts) // 2) * 2)  # even (fp32r free-dim rule)

    in_maps = _pack(xf, idxs, counts, cap, wg, bg, wu, bu, wd, bd)
    return in_maps, idxs, ws, counts, cap, aux_loss


def _retile_up(w):
    """[H, I] -> [KI//2, P, KH, 256]: block j contiguous, 8KB partition lines."""
    w = np.asarray(w, dtype=np.float32)
    return np.ascontiguousarray(
        w.reshape(KH, P, KI // 2, 2 * P).transpose(2, 1, 0, 3)
    )


def _retile_dn(w):
    """[I, H] -> [KH//2, P, KI, 256]."""
    w = np.asarray(w, dtype=np.float32)
    return np.ascontiguousarray(
        w.reshape(KI, P, KH // 2, 2 * P).transpose(2, 1, 0, 3)
    )


def _pack(xf, idxs, counts, cap, wg, bg, wu, bu, wd, bd):
    in_maps = []
    for e in range(E):
        xt = np.zeros((H, cap), np.float32)
        xt[:, : counts[e]] = xf[idxs[e]].T
        in_maps.append(
            {
                "xt": xt,
                "wg": _retile_up(np.asarray(wg)[e]),
                "wu": _retile_up(np.asarray(wu)[e]),
                "wd": _retile_dn(np.asarray(wd)[e]),
                "bg": np.ascontiguousarray(np.asarray(bg)[e], dtype=np.float32),
                "bu": np.ascontiguousarray(np.asarray(bu)[e], dtype=np.float32),
                "bd": np.ascontiguousarray(np.asarray(bd)[e], dtype=np.float32),
            }
        )
    return in_maps


def combine(results, idxs, ws, counts):
    acc = np.zeros((N, H), np.float32)
    for e in range(E):
        y = results[e]["yt"][:, : counts[e]].T  # [cnt, H]
        acc[idxs[e]] += y * ws[e][:, None]
    return acc.reshape(B, S, H)


CAP_MAX = 1280  # largest per-round capacity that fits the SBUF layout


def kernel(hidden_states, w_router, wg, bg, wu, bu, wd, bd):
    from concourse.bass_utils import run_bass_kernel_spmd

    in_maps, idxs, ws, counts, cap, aux_loss = prepare(
        hidden_states, w_router, wg, bg, wu, bu, wd, bd
    )
    cores = list(range(NCORES))

    if cap <= CAP_MAX:
        if cap not in _cache:
            _cache[cap] = _build(cap)
        res = run_bass_kernel_spmd(_cache[cap], in_maps, core_ids=cores)
        return combine(res.results, idxs, ws, counts), aux_loss

    # extreme routing skew: process each expert's tokens in multiple rounds
    xf = np.ascontiguousarray(
        np.asarray(hidden_states).reshape(N, H), dtype=np.float32
    )
    acc = np.zeros((N, H), np.float32)
    rounds = -(-max(counts) // CAP_MAX)
    for r in range(rounds):
        sub_idxs = [ix[r * CAP_MAX : (r + 1) * CAP_MAX] for ix in idxs]
        sub_ws = [w[r * CAP_MAX : (r + 1) * CAP_MAX] for w in ws]
        sub_counts = [len(ix) for ix in sub_idxs]
        cap_r = max(256, -(-max(sub_counts) // 2) * 2)
        if cap_r not in _cache:
            _cache[cap_r] = _build(cap_r)
        maps_r = _pack(xf, sub_idxs, sub_counts, cap_r, wg, bg, wu, bu, wd, bd)
        res = run_bass_kernel_spmd(_cache[cap_r], maps_r, core_ids=cores)
        acc += combine(res.results, sub_idxs, sub_ws, sub_counts).reshape(N, H)
    return acc.reshape(B, S, H), aux_loss
